# revision 8
# baseline (speedup 1.0000x reference)
"""Self-contained Trainium2 Bass kernel for nn_Attention_11836929868027 (v2).

Causal GQA attention prefill (B=2, T=1024, D=4096, 32 q heads / 8 kv heads,
head_dim 128) with per-head RMSNorm on q/k, RoPE, empty kv cache.

Sharding: tensor-parallel over kv-head groups across 8 NeuronCores. Core c
owns kv head c and q heads 4c..4c+3. Each core computes its heads'
projections, attention and a partial o_proj over the full emb_dim; the host
sums the 8 fp32 partials.

v2 vs v1:
- QKV projection emits Q^T/K^T directly (stationary = weight slice,
  streaming = x^T) so no per-chunk PE transposes; RMSNorm reduction over
  the head axis (now partitions) is a single all-ones matmul that also
  broadcasts, rope runs as cross-partition DVE multiplies.
- Startup DMA interleaved per contraction chunk so the PE trails the DMA.
- Softmax normalize bounces the raw denominator row through DRAM and takes
  the reciprocal after the partition-broadcast (128 lanes, approx-fast).
- Projection tiles for batch-1 tokens and o_proj chunks are emitted inside
  the attention gaps so the PE never idles on the exp/normalize chains.
"""

import math

import numpy as np
import ml_dtypes

BF = ml_dtypes.bfloat16

B, T, S = 2, 1024, 2048
D, N, KH, H = 4096, 32, 8, 128
G = N // KH          # 4 q heads per kv head / core
BT = B * T           # 2048 tokens
E = G * H            # 512 q columns per core
DC = D // 128        # 32 contraction chunks
NTC = BT // 128      # 16 token chunks
NQ = BT // 512       # 4 token quarters
EPS = 1e-6
ROPE_THETA = 1e6
NCORES = 8

_CACHE = {}


def _build():
    import concourse.bass as bass
    import concourse.mybir as mybir
    import concourse.tile as tile
    from concourse import bacc
    from concourse.masks import make_identity

    fp32 = mybir.dt.float32
    bf16 = mybir.dt.bfloat16
    MUL = mybir.AluOpType.mult
    ADD = mybir.AluOpType.add
    SUB = mybir.AluOpType.subtract
    AF = mybir.ActivationFunctionType

    nc = bacc.Bacc("TRN2", target_bir_lowering=False, num_devices=NCORES)

    xq_d = nc.declare_dram_parameter("xq", [NQ, 128, DC, 512], bf16, False)
    wqkv_d = nc.declare_dram_parameter("wqkv", [128, DC, E + 2 * H], bf16, False)
    wo_d = nc.declare_dram_parameter("wo", [128, G, D], bf16, False)
    cos_d = nc.declare_dram_parameter("cosT", [128, BT], fp32, False)
    sin_d = nc.declare_dram_parameter("sinT", [128, BT], fp32, False)
    qsc_d = nc.declare_dram_parameter("qscale", [128, 1], fp32, False)
    ksc_d = nc.declare_dram_parameter("kscale", [128, 1], fp32, False)
    mask_d = nc.declare_dram_parameter("maskT", [128, 1024], bf16, False)
    out_d = nc.declare_dram_parameter("out", [BT, D], fp32, True)

    inv_sqrt_h = float(1.0 / math.sqrt(H))

    with tile.TileContext(nc) as tc:
        with (
            tc.tile_pool(name="persist", bufs=1) as pp,
            tc.tile_pool(name="ps", bufs=8, space="PSUM") as ps,
            tc.tile_pool(name="rows", bufs=1) as rows,
            tc.tile_pool(name="p2e", bufs=9) as p2e,
            tc.tile_pool(name="p2t", bufs=1) as p2t,
            tc.tile_pool(name="bcd", bufs=4, space="DRAM") as bcd,
        ):
            # ---- persistent SBUF tensors ----
            QT_sb = pp.tile([128, G, BT], bf16, name="QT_sb")
            KT_sb = pp.tile([128, BT], bf16, name="KT_sb")
            V_sb = pp.tile([128, NTC, H], bf16, name="V_sb")
            OT_sb = pp.tile([128, G, BT], bf16, name="OT_sb")
            qsc_sb = pp.tile([128, 1], fp32, name="qsc_sb")
            ksc_sb = pp.tile([128, 1], fp32, name="ksc_sb")
            mask_sb = pp.tile([128, 1024], bf16, name="mask_sb")
            ones_bf = pp.tile([128, 1], bf16, name="ones_bf")
            onesM = pp.tile([128, 128], bf16, name="onesM")
            ident = pp.tile([128, 128], bf16, name="ident")
            eps_sb = pp.tile([128, 1], fp32, name="eps_sb")

            sel4 = pp.tile([128, 128], bf16, name="sel4")

            nc.vector.memset(ones_bf[:], 1.0)
            nc.vector.memset(onesM[:], 1.0)
            nc.vector.memset(eps_sb[:], EPS)
            nc.vector.memset(sel4[:], 0.0)
            for r in (0, 32, 64, 96):
                nc.vector.memset(sel4[r:r + 1, :], 1.0)
            make_identity(nc, ident[:])
            # clear all PSUM banks once: stale bits from a previous NEFF can
            # decode as NaN, and the Sp4 select-matmul reads whole banks
            for _zi in range(8):
                _zt = ps.tile([128, 512], fp32, name="zt", tag="ps")
                nc.vector.memset(_zt[:], 0.0)

            # =========== Phase 1: QKV projection (transposed out) ===========
            ctx_p1rest = tc.tile_pool(name="p1c", bufs=1)
            p1c = ctx_p1rest.__enter__()
            ctx_p1t = tc.tile_pool(name="p1t", bufs=1)
            p1t = ctx_p1t.__enter__()
            ctx_p1f = tc.tile_pool(name="p1f", bufs=5)
            p1f = ctx_p1f.__enter__()
            ctx_p1wx = tc.tile_pool(name="p1w", bufs=1)
            p1w = ctx_p1wx.__enter__()
            ctx_p1x = tc.tile_pool(name="p1x", bufs=2)
            p1x = ctx_p1x.__enter__()
            if True:
                wqkv_sb = p1w.tile([128, DC, E + 2 * H], bf16, name="wqkv_sb")
                cos_sb = p1c.tile([128, BT], fp32, name="cos_sb")
                sin_sb = p1c.tile([128, BT], fp32, name="sin_sb")

                xts = [None] * NQ

                def queue_xq_dma(tq):
                    xt = p1x.tile([128, DC, 512], bf16, name=f"xq{tq}", tag="xq")
                    for k in range(4):
                        nc.sync.dma_start(
                            out=xt[:, 8 * k:8 * (k + 1), :],
                            in_=xq_d[tq, :, 8 * k:8 * (k + 1), :],
                        )
                    xts[tq] = xt

                # tq0: per-d interleaved DMA so the PE can trail the DMA
                xt0 = p1x.tile([128, DC, 512], bf16, name="xq0", tag="xq")
                xts[0] = xt0
                for d in range(DC):
                    nc.sync.dma_start(
                        out=wqkv_sb[:, d:d + 1, :], in_=wqkv_d[:, d:d + 1, :]
                    )
                    nc.sync.dma_start(
                        out=xt0[:, d:d + 1, :], in_=xq_d[0, :, d:d + 1, :]
                    )

                def emit_proj_mms(tq, hb, d0=0, d1=DC, P=None):
                    """One [128,512] projection tile: accumulating MMs."""
                    if P is None:
                        P = ps.tile([128, 512], fp32, name=f"P{tq}{hb}", tag="ps")
                    for d in range(d0, d1):
                        nc.tensor.matmul(
                            P[:],
                            wqkv_sb[:, d, hb * 128:(hb + 1) * 128],
                            xts[tq][:, d, :],
                            start=(d == 0), stop=(d == DC - 1),
                        )
                    return P

                def emit_head(tq, hb, P):
                    """Drain a q/k projection tile out of PSUM (bf16)."""
                    pf = p1f.tile([128, 512], bf16, name="pf", tag="pf")
                    nc.vector.tensor_copy(pf[:], P[:])
                    return pf

                def emit_tail(tq, hb, pf):
                    """RMSNorm + rope from the SBUF copy into QT/KT."""
                    t0 = tq * 512
                    sq = p1t.tile([128, 512], bf16, name="sq", tag="sq")
                    nc.gpsimd.tensor_tensor(sq[:], pf[:], pf[:], MUL)
                    bc = ps.tile([128, 512], fp32, name="bc", tag="ps")
                    nc.tensor.matmul(
                        bc[:], onesM[:], sq[:], start=True, stop=True
                    )
                    srt = p1t.tile([128, 512], fp32, name="srt", tag="srt")
                    nc.scalar.activation(
                        srt[:], bc[:], AF.Sqrt,
                        bias=eps_sb[:], scale=float(1.0 / H),
                    )
                    rstd = p1t.tile([128, 512], fp32, name="rstd", tag="rstd")
                    nc.vector.reciprocal_approx_fast(rstd[:], srt[:])
                    qn = p1t.tile([128, 512], bf16, name="qn", tag="qn")
                    sc = qsc_sb if hb < 4 else ksc_sb
                    nc.vector.scalar_tensor_tensor(
                        qn[:], pf[:], sc[:], rstd[:], MUL, MUL
                    )
                    # rope: rotate halves across partitions via DMA,
                    # then partition-aligned multiplies with signed trig
                    qrot = p1t.tile([128, 512], bf16, name="qrot", tag="qrot")
                    nc.gpsimd.dma_start(out=qrot[0:64, :], in_=qn[64:128, :])
                    nc.gpsimd.dma_start(out=qrot[64:128, :], in_=qn[0:64, :])
                    cs = cos_sb[:, t0:t0 + 512]
                    sn = sin_sb[:, t0:t0 + 512]
                    m1 = p1t.tile([128, 512], bf16, name="m1", tag="m1")
                    nc.vector.tensor_tensor(m1[:], qn[:], cs, MUL)
                    dest = (
                        QT_sb[:, hb, t0:t0 + 512] if hb < 4
                        else KT_sb[:, t0:t0 + 512]
                    )
                    nc.vector.tensor_tensor(dest, qrot[:], sn, MUL)
                    nc.vector.tensor_tensor(dest, m1[:], dest, ADD)

                def emit_pp(tq, hb, P):
                    """Postprocess one projection tile into QT/KT/V."""
                    if hb < 5:
                        emit_tail(tq, hb, emit_head(tq, hb, P))
                    else:
                        # v head: cast then transpose chunks into [keys, h]
                        vt = p1t.tile([128, 512], bf16, name="vt", tag="vt")
                        nc.vector.tensor_copy(vt[:], P[:])
                        for j in range(4):
                            tp = ps.tile([128, 128], bf16, name="tp", tag="ps")
                            nc.tensor.transpose(
                                tp[:], vt[:, j * 128:(j + 1) * 128], ident[:]
                            )
                            nc.vector.tensor_copy(
                                V_sb[:, tq * 4 + j, :], tp[:]
                            )

                # ================= attention machinery =================
                def emit_logits_seg(b, g, t0):
                    col0 = b * T + t0
                    nS = (t0 + 512) // 128
                    eTs = []
                    for c in range(nS):
                        off = max(0, 128 * c - t0)
                        Lp = ps.tile([128, 512], fp32, name="Lp", tag="ps")
                        nc.tensor.matmul(
                            Lp[:, off:512],
                            KT_sb[:, b * T + c * 128:b * T + (c + 1) * 128],
                            QT_sb[:, g, col0 + off:col0 + 512],
                            start=True, stop=True,
                        )
                        eT = p2e.tile([128, 512], bf16, name="eT", tag="eT")
                        nc.scalar.activation(
                            eT[:, off:512], Lp[:, off:512],
                            AF.Exp, scale=inv_sqrt_h,
                        )
                        if 128 * c + 127 > t0:
                            u0 = 512 + t0 - 128 * c
                            nc.vector.tensor_tensor(
                                eT[:, off:512], eT[:, off:512],
                                mask_sb[:, u0 + off:u0 + 512], MUL,
                            )
                        eTs.append((eT, off))
                    return eTs

                def emit_av_seg(b, g, t0, eTs):
                    col0 = b * T + t0
                    nS = (t0 + 512) // 128
                    OTp = ps.tile([128, 512], fp32, name="OTp", tag="ps")
                    Sp4 = ps.tile([128, 512], fp32, name="Sp4", tag="ps")
                    nc.vector.memset(Sp4[:], 0.0)
                    for c in range(nS):
                        sc = b * (T // 128) + c
                        eT, off = eTs[c]
                        nc.tensor.matmul(
                            OTp[:, off:512], V_sb[:, sc, :],
                            eT[:, off:512],
                            start=(c == 0), stop=(c == nS - 1),
                        )
                    # denominator: four concurrent rank-1 sums on separate
                    # column groups, then a select-ones matmul that both
                    # reduces the four rows and broadcasts to all partitions
                    for c in range(nS):
                        eT, off = eTs[c]
                        j = c % 4
                        nc.tensor.matmul(
                            Sp4[32 * j:32 * j + 1, off:512], ones_bf[:],
                            eT[:, off:512],
                            start=(c < 4), stop=(c >= nS - 4),
                            tile_position=(0, 32 * j),
                        )
                    spc = p2t.tile([128, 512], bf16, name="spc", tag="bcs")
                    nc.vector.tensor_copy(spc[:], Sp4[:])
                    bc2 = ps.tile([128, 512], fp32, name="bc2", tag="ps")
                    nc.tensor.matmul(
                        bc2[:], sel4[:], spc[:], start=True, stop=True
                    )
                    rec = p2t.tile([128, 512], fp32, name="rec", tag="rec")
                    nc.vector.reciprocal_approx_fast(rec[:], bc2[:])
                    nc.vector.tensor_tensor(
                        OT_sb[:, g, col0:col0 + 512], OTp[:], rec[:], MUL
                    )

                # tq0: d-outer so each arriving chunk is consumed immediately
                P0 = [
                    ps.tile([128, 512], fp32, name=f"P0{hb}", tag="ps")
                    for hb in range(6)
                ]
                for d in range(DC):
                    for hb in range(6):
                        nc.tensor.matmul(
                            P0[hb][:],
                            wqkv_sb[:, d, hb * 128:(hb + 1) * 128],
                            xt0[:, d, :],
                            start=(d == 0), stop=(d == DC - 1),
                        )
                nc.sync.dma_start(out=qsc_sb[:], in_=qsc_d[:])
                nc.sync.dma_start(out=ksc_sb[:], in_=ksc_d[:])
                nc.sync.dma_start(out=mask_sb[:], in_=mask_d[:])
                queue_xq_dma(1)
                nc.sync.dma_start(out=cos_sb[:], in_=cos_d[:])
                nc.sync.dma_start(out=sin_sb[:], in_=sin_d[:])

                # tq1 blocks interleaved with tq0 postprocess
                P1 = [None] * 6
                for i in range(6):
                    if i < 5:
                        P1[i] = emit_proj_mms(1, i)
                        emit_pp(0, i, P0[i])
                    else:
                        emit_pp(0, 5, P0[5])
                        P1[5] = emit_proj_mms(1, 5)
                queue_xq_dma(2)
                # tq2 blocks interleaved with tq1 postprocess
                P2 = [None] * 6
                for i in range(6):
                    if i < 5:
                        P2[i] = emit_proj_mms(2, i)
                        emit_pp(1, i, P1[i])
                    else:
                        emit_pp(1, 5, P1[5])
                        P2[5] = emit_proj_mms(2, 5)
                queue_xq_dma(3)
                # wo goes into the tq2 xq slot (free after its last matmul),
                # so it is resident well before the first o_proj half
                wo_sb = p1x.tile([128, G, D], bf16, name="wo_sb", tag="xq")
                for k in range(2):
                    nc.sync.dma_start(
                        out=wo_sb[:, 2 * k:2 * (k + 1), :],
                        in_=wo_d[:, 2 * k:2 * (k + 1), :],
                    )
                # pre-exp region: tq2 postprocess paired with tq3 block
                # halves so the PE stays fed while the norm chains drain;
                # all ACT sqrts here run before any attention exp
                emit_pp(2, 0, P2[0])
                emit_pp(2, 1, P2[1])
                P30 = emit_proj_mms(3, 0, 0, 16)
                emit_pp(2, 2, P2[2])
                emit_proj_mms(3, 0, 16, DC, P30)
                emit_pp(2, 3, P2[3])
                P31 = emit_proj_mms(3, 1, 0, 16)
                emit_pp(2, 4, P2[4])
                emit_proj_mms(3, 1, 16, DC, P31)
                emit_pp(2, 5, P2[5])
                pf_tail = [(0, emit_head(3, 0, P30)), (1, emit_head(3, 1, P31))]

                # attention b0: fills are pure matmul blocks (no ACT), the
                # deferred norm tails run clustered after the last b0 exp
                for g in range(G):
                    hb = g + 2           # tq3 blocks g2,g3,k,v
                    eTs0 = emit_logits_seg(0, g, 0)
                    Ph = emit_proj_mms(3, hb, 0, 16)
                    emit_av_seg(0, g, 0, eTs0)
                    eTs1 = emit_logits_seg(0, g, 512)
                    emit_proj_mms(3, hb, 16, DC, Ph)
                    if hb == 5:
                        emit_pp(3, 5, Ph)   # v: transposes only, ACT-free
                    else:
                        pf_tail.append((hb, emit_head(3, hb, Ph)))
                    emit_av_seg(0, g, 512, eTs1)

            # ======= Phase 2: attention b1 + o_proj, then o_proj tail =======
            with (
                tc.tile_pool(name="p3o", bufs=4) as p3o,
            ):
                def emit_p3_half(tci, half, alt, use_act=False):
                    tcol = tci * 128
                    pso = []
                    for dh in range(4 * half, 4 * half + 4):
                        p = ps.tile([128, 512], fp32, name="pso", tag="ps")
                        pso.append((dh, p))
                    for g in range(G):
                        for dh, p in pso:
                            nc.tensor.matmul(
                                p[:],
                                OT_sb[:, g, tcol:tcol + 128],
                                wo_sb[:, g, dh * 512:(dh + 1) * 512],
                                start=(g == 0), stop=(g == G - 1),
                            )
                    for j, (dh, p) in enumerate(pso):
                        ob = p3o.tile([128, 512], fp32, name="ob", tag="ob")
                        if use_act and (alt + j) % 2 == 1:
                            nc.scalar.copy(ob[:], p[:])
                        else:
                            nc.vector.tensor_copy(ob[:], p[:])
                        nc.sync.dma_start(
                            out=out_d[tcol:tcol + 128, dh * 512:(dh + 1) * 512],
                            in_=ob[:],
                        )

                p3_queue = [
                    (tci, half) for tci in range(NTC) for half in range(2)
                ]
                p3_done = 0

                # cluster 2: deferred tq3 norm tails (ACT sqrts grouped),
                # interleaved with the first o_proj halves as PE fill
                for i, (hb, pf) in enumerate(pf_tail):
                    emit_tail(3, hb, pf)
                    if i < 4:
                        tci_h = p3_queue[p3_done]
                        emit_p3_half(tci_h[0], tci_h[1], p3_done)
                        p3_done += 1

                # b0 chunk halves only until every b1 t0-segment is done
                # (b1 chunks need all four heads of the b1 t0 attention)
                for g in range(G):
                    for t0 in (0, 512):
                        eTs = emit_logits_seg(1, g, t0)
                        cap = 16 if (g < 3 or t0 == 512) else p3_done
                        for _ in range(2):
                            if p3_done < cap or (g == 3 and t0 == 512
                                                 and p3_done < 18):
                                tci_h = p3_queue[p3_done]
                                emit_p3_half(tci_h[0], tci_h[1], p3_done)
                                p3_done += 1
                        emit_av_seg(1, g, t0, eTs)

                while p3_done < len(p3_queue):
                    tci_h = p3_queue[p3_done]
                    emit_p3_half(tci_h[0], tci_h[1], p3_done, use_act=True)
                    p3_done += 1

            ctx_p1x.__exit__(None, None, None)
            ctx_p1wx.__exit__(None, None, None)
            ctx_p1f.__exit__(None, None, None)
            ctx_p1t.__exit__(None, None, None)
            ctx_p1rest.__exit__(None, None, None)

    nc.compile()
    return nc


def _prep_inputs(x, wq, wk, wv, wo, q_scale, k_scale, segment_ids):
    """Host-side shard prep. Returns in_maps for the 8 cores."""
    x2 = np.ascontiguousarray(np.asarray(x, dtype=np.float32).reshape(BT, D))
    xT = x2.T.astype(BF)                                   # [D, BT]
    xq = np.ascontiguousarray(
        xT.reshape(DC, 128, NQ, 512).transpose(2, 1, 0, 3)
    )                                                      # [NQ,128,DC,512]

    seg = np.asarray(segment_ids)
    first = np.argmax(seg, axis=1)
    pos = np.where(
        seg != 0, np.arange(T, dtype=np.int64)[None, :] - first[:, None], 2 ** 30
    )
    fraction = np.arange(0, H, 2, dtype=np.float64) / H
    inv_freq = 1.0 / (ROPE_THETA ** fraction)
    sinus = pos.reshape(-1).astype(np.float64)[:, None] * inv_freq[None, :]
    cosf = np.cos(sinus).astype(np.float32)                # [BT, 64]
    sinf = np.sin(sinus).astype(np.float32)
    cosT = np.ascontiguousarray(np.concatenate([cosf.T, cosf.T], axis=0))
    sinT = np.ascontiguousarray(np.concatenate([-sinf.T, sinf.T], axis=0))

    qsc = np.ascontiguousarray(np.asarray(q_scale, np.float32).reshape(128, 1))
    ksc = np.ascontiguousarray(np.asarray(k_scale, np.float32).reshape(128, 1))

    su = np.arange(128)[:, None] <= (np.arange(1024)[None, :] - 512)
    maskT = su.astype(BF)                                  # [128, 1024]

    wq2 = np.asarray(wq, np.float32).reshape(D, N * H)
    wk2 = np.asarray(wk, np.float32).reshape(D, KH * H)
    wv2 = np.asarray(wv, np.float32).reshape(D, KH * H)
    wo2 = np.asarray(wo, np.float32)                       # [N, H, D]

    in_maps = []
    for c in range(NCORES):
        wqkv = np.concatenate(
            [
                wq2[:, c * E:(c + 1) * E],
                wk2[:, c * H:(c + 1) * H],
                wv2[:, c * H:(c + 1) * H],
            ],
            axis=1,
        ).astype(BF)                                       # [D, 768]
        wqkvt = np.ascontiguousarray(
            wqkv.reshape(DC, 128, E + 2 * H).transpose(1, 0, 2)
        )                                                  # [128, DC, 768]
        woc = wo2[c * G:(c + 1) * G].astype(BF)            # [G, H, D]
        wot = np.ascontiguousarray(woc.transpose(1, 0, 2))  # [128, G, D]
        in_maps.append(
            {
                "xq": xq,
                "wqkv": wqkvt,
                "wo": wot,
                "cosT": cosT,
                "sinT": sinT,
                "qscale": qsc,
                "kscale": ksc,
                "maskT": maskT,
            }
        )
    return in_maps


def kernel(x, wq, wk, wv, wo, q_scale, k_scale, k_cache, v_cache,
           segment_ids, num_right_pads=0, **_unused):
    from concourse.bass_utils import run_bass_kernel_spmd

    if "nc" not in _CACHE:
        _CACHE["nc"] = _build()
    nc = _CACHE["nc"]

    in_maps = _prep_inputs(x, wq, wk, wv, wo, q_scale, k_scale, segment_ids)
    res = run_bass_kernel_spmd(nc, in_maps, core_ids=list(range(NCORES)))
    total = np.zeros((BT, D), np.float32)
    for c in range(NCORES):
        total += np.asarray(res.results[c]["out"], dtype=np.float32)
    return total.reshape(B, T, D)


# revision 9
# speedup vs baseline: 1.0746x; 1.0746x over previous
"""Self-contained Trainium2 Bass kernel for nn_Attention_11836929868027 (v2).

Causal GQA attention prefill (B=2, T=1024, D=4096, 32 q heads / 8 kv heads,
head_dim 128) with per-head RMSNorm on q/k, RoPE, empty kv cache.

Sharding: tensor-parallel over kv-head groups across 8 NeuronCores. Core c
owns kv head c and q heads 4c..4c+3. Each core computes its heads'
projections, attention and a partial o_proj over the full emb_dim; the host
sums the 8 fp32 partials.

v2 vs v1:
- QKV projection emits Q^T/K^T directly (stationary = weight slice,
  streaming = x^T) so no per-chunk PE transposes; RMSNorm reduction over
  the head axis (now partitions) is a single all-ones matmul that also
  broadcasts, rope runs as cross-partition DVE multiplies.
- Startup DMA interleaved per contraction chunk so the PE trails the DMA.
- Softmax normalize bounces the raw denominator row through DRAM and takes
  the reciprocal after the partition-broadcast (128 lanes, approx-fast).
- Projection tiles for batch-1 tokens and o_proj chunks are emitted inside
  the attention gaps so the PE never idles on the exp/normalize chains.
"""

import math

import numpy as np
import ml_dtypes

BF = ml_dtypes.bfloat16

B, T, S = 2, 1024, 2048
D, N, KH, H = 4096, 32, 8, 128
G = N // KH          # 4 q heads per kv head / core
BT = B * T           # 2048 tokens
E = G * H            # 512 q columns per core
DC = D // 128        # 32 contraction chunks
NTC = BT // 128      # 16 token chunks
NQ = BT // 512       # 4 token quarters
EPS = 1e-6
ROPE_THETA = 1e6
NCORES = 8

_CACHE = {}


def _build():
    import concourse.bass as bass
    import concourse.mybir as mybir
    import concourse.tile as tile
    from concourse import bacc
    from concourse.masks import make_identity

    fp32 = mybir.dt.float32
    bf16 = mybir.dt.bfloat16
    MUL = mybir.AluOpType.mult
    ADD = mybir.AluOpType.add
    SUB = mybir.AluOpType.subtract
    AF = mybir.ActivationFunctionType

    nc = bacc.Bacc("TRN2", target_bir_lowering=False, num_devices=NCORES)

    xq_d = nc.declare_dram_parameter("xq", [NQ, 128, DC, 512], bf16, False)
    wqkv_d = nc.declare_dram_parameter("wqkv", [128, DC, E + 2 * H], bf16, False)
    wo_d = nc.declare_dram_parameter("wo", [128, G, D], bf16, False)
    cos_d = nc.declare_dram_parameter("cosT", [128, BT], fp32, False)
    sin_d = nc.declare_dram_parameter("sinT", [128, BT], fp32, False)
    qsc_d = nc.declare_dram_parameter("qscale", [128, 1], fp32, False)
    ksc_d = nc.declare_dram_parameter("kscale", [128, 1], fp32, False)
    mask_d = nc.declare_dram_parameter("maskT", [128, 1024], bf16, False)
    out_d = nc.declare_dram_parameter("out", [BT, D], fp32, True)

    inv_sqrt_h = float(1.0 / math.sqrt(H))

    with tile.TileContext(nc) as tc:
        with (
            tc.tile_pool(name="persist", bufs=1) as pp,
            tc.tile_pool(name="ps", bufs=8, space="PSUM") as ps,
            tc.tile_pool(name="rows", bufs=1) as rows,
            tc.tile_pool(name="p2e", bufs=9) as p2e,
            tc.tile_pool(name="p2t", bufs=1) as p2t,
            tc.tile_pool(name="bcd", bufs=4, space="DRAM") as bcd,
        ):
            # ---- persistent SBUF tensors ----
            QT_sb = pp.tile([128, G, BT], bf16, name="QT_sb")
            KT_sb = pp.tile([128, BT], bf16, name="KT_sb")
            V_sb = pp.tile([128, NTC, H], bf16, name="V_sb")
            OT_sb = pp.tile([128, G, BT], bf16, name="OT_sb")
            qsc_sb = pp.tile([128, 1], fp32, name="qsc_sb")
            ksc_sb = pp.tile([128, 1], fp32, name="ksc_sb")
            mask_sb = pp.tile([128, 1024], bf16, name="mask_sb")
            ones_bf = pp.tile([128, 1], bf16, name="ones_bf")
            onesM = pp.tile([128, 128], bf16, name="onesM")
            ident = pp.tile([128, 128], bf16, name="ident")
            eps_sb = pp.tile([128, 1], fp32, name="eps_sb")

            sel4 = pp.tile([128, 128], bf16, name="sel4")

            nc.vector.memset(ones_bf[:], 1.0)
            nc.vector.memset(onesM[:], 1.0)
            nc.vector.memset(eps_sb[:], EPS)
            nc.vector.memset(sel4[:], 0.0)
            for r in (0, 32, 64, 96):
                nc.vector.memset(sel4[r:r + 1, :], 1.0)
            make_identity(nc, ident[:])
            # clear all PSUM banks once: stale bits from a previous NEFF can
            # decode as NaN, and the Sp4 select-matmul reads whole banks
            for _zi in range(8):
                _zt = ps.tile([128, 512], fp32, name="zt", tag="ps")
                nc.vector.memset(_zt[:], 0.0)

            # =========== Phase 1: QKV projection (transposed out) ===========
            ctx_p1rest = tc.tile_pool(name="p1c", bufs=1)
            p1c = ctx_p1rest.__enter__()
            ctx_p1t = tc.tile_pool(name="p1t", bufs=1)
            p1t = ctx_p1t.__enter__()
            ctx_p1f = tc.tile_pool(name="p1f", bufs=5)
            p1f = ctx_p1f.__enter__()
            ctx_p1wx = tc.tile_pool(name="p1w", bufs=1)
            p1w = ctx_p1wx.__enter__()
            ctx_p1x = tc.tile_pool(name="p1x", bufs=2)
            p1x = ctx_p1x.__enter__()
            if True:
                wqkv_sb = p1w.tile([128, DC, E + 2 * H], bf16, name="wqkv_sb")
                cos_sb = p1c.tile([128, BT], fp32, name="cos_sb")
                sin_sb = p1c.tile([128, BT], fp32, name="sin_sb")

                xts = [None] * NQ

                def queue_xq_dma(tq):
                    xt = p1x.tile([128, DC, 512], bf16, name=f"xq{tq}", tag="xq")
                    for k in range(4):
                        nc.sync.dma_start(
                            out=xt[:, 8 * k:8 * (k + 1), :],
                            in_=xq_d[tq, :, 8 * k:8 * (k + 1), :],
                        )
                    xts[tq] = xt

                # tq0: per-d interleaved DMA so the PE can trail the DMA
                xt0 = p1x.tile([128, DC, 512], bf16, name="xq0", tag="xq")
                xts[0] = xt0
                for d in range(DC):
                    nc.sync.dma_start(
                        out=wqkv_sb[:, d:d + 1, :], in_=wqkv_d[:, d:d + 1, :]
                    )
                    nc.sync.dma_start(
                        out=xt0[:, d:d + 1, :], in_=xq_d[0, :, d:d + 1, :]
                    )

                def emit_proj_mms(tq, hb, d0=0, d1=DC, P=None):
                    """One [128,512] projection tile: accumulating MMs."""
                    if P is None:
                        P = ps.tile([128, 512], fp32, name=f"P{tq}{hb}", tag="ps")
                    for d in range(d0, d1):
                        nc.tensor.matmul(
                            P[:],
                            wqkv_sb[:, d, hb * 128:(hb + 1) * 128],
                            xts[tq][:, d, :],
                            start=(d == 0), stop=(d == DC - 1),
                        )
                    return P

                def emit_head(tq, hb, P):
                    """Drain a q/k projection tile out of PSUM (bf16)."""
                    pf = p1f.tile([128, 512], bf16, name="pf", tag="pf")
                    nc.vector.tensor_copy(pf[:], P[:])
                    return pf

                def emit_tail(tq, hb, pf):
                    """RMSNorm + rope from the SBUF copy into QT/KT."""
                    t0 = tq * 512
                    sq = p1t.tile([128, 512], bf16, name="sq", tag="sq")
                    nc.gpsimd.tensor_tensor(sq[:], pf[:], pf[:], MUL)
                    bc = ps.tile([128, 512], fp32, name="bc", tag="ps")
                    nc.tensor.matmul(
                        bc[:], onesM[:], sq[:], start=True, stop=True
                    )
                    srt = p1t.tile([128, 512], fp32, name="srt", tag="srt")
                    nc.scalar.activation(
                        srt[:], bc[:], AF.Sqrt,
                        bias=eps_sb[:], scale=float(1.0 / H),
                    )
                    rstd = p1t.tile([128, 512], fp32, name="rstd", tag="rstd")
                    nc.vector.reciprocal_approx_fast(rstd[:], srt[:])
                    qn = p1t.tile([128, 512], bf16, name="qn", tag="qn")
                    sc = qsc_sb if hb < 4 else ksc_sb
                    nc.vector.scalar_tensor_tensor(
                        qn[:], pf[:], sc[:], rstd[:], MUL, MUL
                    )
                    # rope: rotate halves across partitions via DMA,
                    # then partition-aligned multiplies with signed trig
                    qrot = p1t.tile([128, 512], bf16, name="qrot", tag="qrot")
                    nc.gpsimd.dma_start(out=qrot[0:64, :], in_=qn[64:128, :])
                    nc.gpsimd.dma_start(out=qrot[64:128, :], in_=qn[0:64, :])
                    cs = cos_sb[:, t0:t0 + 512]
                    sn = sin_sb[:, t0:t0 + 512]
                    m1 = p1t.tile([128, 512], bf16, name="m1", tag="m1")
                    nc.vector.tensor_tensor(m1[:], qn[:], cs, MUL)
                    dest = (
                        QT_sb[:, hb, t0:t0 + 512] if hb < 4
                        else KT_sb[:, t0:t0 + 512]
                    )
                    nc.vector.tensor_tensor(dest, qrot[:], sn, MUL)
                    nc.vector.tensor_tensor(dest, m1[:], dest, ADD)

                def emit_pp(tq, hb, P):
                    """Postprocess one projection tile into QT/KT/V."""
                    if hb < 5:
                        emit_tail(tq, hb, emit_head(tq, hb, P))
                    else:
                        # v head: cast then transpose chunks into [keys, h]
                        vt = p1t.tile([128, 512], bf16, name="vt", tag="vt")
                        nc.vector.tensor_copy(vt[:], P[:])
                        for j in range(4):
                            tp = ps.tile([128, 128], bf16, name="tp", tag="ps")
                            nc.tensor.transpose(
                                tp[:], vt[:, j * 128:(j + 1) * 128], ident[:]
                            )
                            nc.vector.tensor_copy(
                                V_sb[:, tq * 4 + j, :], tp[:]
                            )

                # ================= attention machinery =================
                def emit_logits_seg(b, g, t0):
                    col0 = b * T + t0
                    nS = (t0 + 512) // 128
                    eTs = []
                    for c in range(nS):
                        off = max(0, 128 * c - t0)
                        Lp = ps.tile([128, 512], fp32, name="Lp", tag="ps")
                        nc.tensor.matmul(
                            Lp[:, off:512],
                            KT_sb[:, b * T + c * 128:b * T + (c + 1) * 128],
                            QT_sb[:, g, col0 + off:col0 + 512],
                            start=True, stop=True,
                        )
                        eT = p2e.tile([128, 512], bf16, name="eT", tag="eT")
                        nc.scalar.activation(
                            eT[:, off:512], Lp[:, off:512],
                            AF.Exp, scale=inv_sqrt_h,
                        )
                        if 128 * c + 127 > t0:
                            u0 = 512 + t0 - 128 * c
                            nc.vector.tensor_tensor(
                                eT[:, off:512], eT[:, off:512],
                                mask_sb[:, u0 + off:u0 + 512], MUL,
                            )
                        eTs.append((eT, off))
                    return eTs

                pending_fin = []

                def flush_fin():
                    while pending_fin:
                        pending_fin.pop(0)()

                def emit_av_seg(b, g, t0, eTs):
                    flush_fin()
                    col0 = b * T + t0
                    nS = (t0 + 512) // 128
                    OTp = ps.tile([128, 512], fp32, name="OTp", tag="ps")
                    Sp4 = ps.tile([128, 512], fp32, name="Sp4", tag="ps")
                    nc.vector.memset(Sp4[:], 0.0)
                    for c in range(nS):
                        sc = b * (T // 128) + c
                        eT, off = eTs[c]
                        nc.tensor.matmul(
                            OTp[:, off:512], V_sb[:, sc, :],
                            eT[:, off:512],
                            start=(c == 0), stop=(c == nS - 1),
                        )
                    # denominator: four concurrent rank-1 sums on separate
                    # column groups, then a select-ones matmul that both
                    # reduces the four rows and broadcasts to all partitions
                    for c in range(nS):
                        eT, off = eTs[c]
                        j = c % 4
                        nc.tensor.matmul(
                            Sp4[32 * j:32 * j + 1, off:512], ones_bf[:],
                            eT[:, off:512],
                            start=(c < 4), stop=(c >= nS - 4),
                            tile_position=(0, 32 * j),
                        )
                    spc = p2t.tile([128, 512], bf16, name="spc", tag="bcs")
                    nc.vector.tensor_copy(spc[:], Sp4[:])

                    def fin():
                        # runs one block later so the PE never waits on spc
                        bc2 = ps.tile([128, 512], fp32, name="bc2", tag="ps")
                        nc.tensor.matmul(
                            bc2[:], sel4[:], spc[:], start=True, stop=True
                        )
                        rec = p2t.tile([128, 512], fp32, name="rec", tag="rec")
                        nc.vector.reciprocal_approx_fast(rec[:], bc2[:])
                        nc.vector.tensor_tensor(
                            OT_sb[:, g, col0:col0 + 512], OTp[:], rec[:], MUL
                        )
                    pending_fin.append(fin)

                # tq0: d-outer so each arriving chunk is consumed immediately
                P0 = [
                    ps.tile([128, 512], fp32, name=f"P0{hb}", tag="ps")
                    for hb in range(6)
                ]
                for d in range(DC):
                    for hb in range(6):
                        nc.tensor.matmul(
                            P0[hb][:],
                            wqkv_sb[:, d, hb * 128:(hb + 1) * 128],
                            xt0[:, d, :],
                            start=(d == 0), stop=(d == DC - 1),
                        )
                nc.sync.dma_start(out=qsc_sb[:], in_=qsc_d[:])
                nc.sync.dma_start(out=ksc_sb[:], in_=ksc_d[:])
                nc.sync.dma_start(out=mask_sb[:], in_=mask_d[:])
                queue_xq_dma(1)
                nc.sync.dma_start(out=cos_sb[:], in_=cos_d[:])
                nc.sync.dma_start(out=sin_sb[:], in_=sin_d[:])

                # tq1 blocks interleaved with tq0 postprocess
                P1 = [None] * 6
                for i in range(6):
                    if i < 5:
                        P1[i] = emit_proj_mms(1, i)
                        emit_pp(0, i, P0[i])
                    else:
                        emit_pp(0, 5, P0[5])
                        P1[5] = emit_proj_mms(1, 5)
                queue_xq_dma(2)
                # tq2 blocks interleaved with tq1 postprocess
                P2 = [None] * 6
                for i in range(6):
                    if i < 5:
                        P2[i] = emit_proj_mms(2, i)
                        emit_pp(1, i, P1[i])
                    else:
                        emit_pp(1, 5, P1[5])
                        P2[5] = emit_proj_mms(2, 5)
                queue_xq_dma(3)
                # wo goes into the tq2 xq slot (free after its last matmul),
                # so it is resident well before the first o_proj half
                wo_sb = p1x.tile([128, G, D], bf16, name="wo_sb", tag="xq")
                for k in range(2):
                    nc.sync.dma_start(
                        out=wo_sb[:, 2 * k:2 * (k + 1), :],
                        in_=wo_d[:, 2 * k:2 * (k + 1), :],
                    )
                # pre-exp region: tq2 postprocess paired with tq3 block
                # halves so the PE stays fed while the norm chains drain;
                # all ACT sqrts here run before any attention exp
                emit_pp(2, 0, P2[0])
                emit_pp(2, 1, P2[1])
                P30 = emit_proj_mms(3, 0, 0, 16)
                emit_pp(2, 2, P2[2])
                emit_proj_mms(3, 0, 16, DC, P30)
                emit_pp(2, 3, P2[3])
                P31 = emit_proj_mms(3, 1, 0, 16)
                emit_pp(2, 4, P2[4])
                emit_proj_mms(3, 1, 16, DC, P31)
                emit_pp(2, 5, P2[5])
                pf_tail = [(0, emit_head(3, 0, P30)), (1, emit_head(3, 1, P31))]

                # attention b0: fills are pure matmul blocks (no ACT), the
                # deferred norm tails run clustered after the last b0 exp
                for g in range(G):
                    hb = g + 2           # tq3 blocks g2,g3,k,v
                    eTs0 = emit_logits_seg(0, g, 0)
                    flush_fin()
                    Ph = emit_proj_mms(3, hb, 0, 16)
                    emit_av_seg(0, g, 0, eTs0)
                    eTs1 = emit_logits_seg(0, g, 512)
                    emit_proj_mms(3, hb, 16, DC, Ph)
                    if hb == 5:
                        emit_pp(3, 5, Ph)   # v: transposes only, ACT-free
                    else:
                        pf_tail.append((hb, emit_head(3, hb, Ph)))
                    emit_av_seg(0, g, 512, eTs1)

            # ======= Phase 2: attention b1 + o_proj, then o_proj tail =======
            with (
                tc.tile_pool(name="p3o", bufs=4) as p3o,
            ):
                def emit_p3_half(tci, half, alt, use_act=False):
                    tcol = tci * 128
                    pso = []
                    for dh in range(4 * half, 4 * half + 4):
                        p = ps.tile([128, 512], fp32, name="pso", tag="ps")
                        pso.append((dh, p))
                    for g in range(G):
                        for dh, p in pso:
                            nc.tensor.matmul(
                                p[:],
                                OT_sb[:, g, tcol:tcol + 128],
                                wo_sb[:, g, dh * 512:(dh + 1) * 512],
                                start=(g == 0), stop=(g == G - 1),
                            )
                    for j, (dh, p) in enumerate(pso):
                        ob = p3o.tile([128, 512], fp32, name="ob", tag="ob")
                        if use_act and (alt + j) % 2 == 1:
                            nc.scalar.copy(ob[:], p[:])
                        else:
                            nc.vector.tensor_copy(ob[:], p[:])
                        nc.sync.dma_start(
                            out=out_d[tcol:tcol + 128, dh * 512:(dh + 1) * 512],
                            in_=ob[:],
                        )

                p3_queue = [
                    (tci, half) for tci in range(NTC) for half in range(2)
                ]
                p3_done = 0

                # cluster 2: deferred tq3 norm tails (ACT sqrts grouped),
                # interleaved with the first o_proj halves as PE fill
                flush_fin()
                for i, (hb, pf) in enumerate(pf_tail):
                    emit_tail(3, hb, pf)
                    if i < 4:
                        tci_h = p3_queue[p3_done]
                        emit_p3_half(tci_h[0], tci_h[1], p3_done, use_act=True)
                        p3_done += 1

                # b0 chunk halves only until every b1 t0-segment is done
                # (b1 chunks need all four heads of the b1 t0 attention)
                for g in range(G):
                    for t0 in (0, 512):
                        eTs = emit_logits_seg(1, g, t0)
                        flush_fin()
                        cap = 16 if (g < 3 or t0 == 512) else p3_done
                        for _ in range(2):
                            if p3_done < cap or (g == 3 and t0 == 512
                                                 and p3_done < 18):
                                tci_h = p3_queue[p3_done]
                                emit_p3_half(tci_h[0], tci_h[1], p3_done)
                                p3_done += 1
                        emit_av_seg(1, g, t0, eTs)

                flush_fin()
                while p3_done < len(p3_queue):
                    tci_h = p3_queue[p3_done]
                    emit_p3_half(tci_h[0], tci_h[1], p3_done, use_act=True)
                    p3_done += 1

            ctx_p1x.__exit__(None, None, None)
            ctx_p1wx.__exit__(None, None, None)
            ctx_p1f.__exit__(None, None, None)
            ctx_p1t.__exit__(None, None, None)
            ctx_p1rest.__exit__(None, None, None)

    nc.compile()
    return nc


def _prep_inputs(x, wq, wk, wv, wo, q_scale, k_scale, segment_ids):
    """Host-side shard prep. Returns in_maps for the 8 cores."""
    x2 = np.ascontiguousarray(np.asarray(x, dtype=np.float32).reshape(BT, D))
    xT = x2.T.astype(BF)                                   # [D, BT]
    xq = np.ascontiguousarray(
        xT.reshape(DC, 128, NQ, 512).transpose(2, 1, 0, 3)
    )                                                      # [NQ,128,DC,512]

    seg = np.asarray(segment_ids)
    first = np.argmax(seg, axis=1)
    pos = np.where(
        seg != 0, np.arange(T, dtype=np.int64)[None, :] - first[:, None], 2 ** 30
    )
    fraction = np.arange(0, H, 2, dtype=np.float64) / H
    inv_freq = 1.0 / (ROPE_THETA ** fraction)
    sinus = pos.reshape(-1).astype(np.float64)[:, None] * inv_freq[None, :]
    cosf = np.cos(sinus).astype(np.float32)                # [BT, 64]
    sinf = np.sin(sinus).astype(np.float32)
    cosT = np.ascontiguousarray(np.concatenate([cosf.T, cosf.T], axis=0))
    sinT = np.ascontiguousarray(np.concatenate([-sinf.T, sinf.T], axis=0))

    qsc = np.ascontiguousarray(np.asarray(q_scale, np.float32).reshape(128, 1))
    ksc = np.ascontiguousarray(np.asarray(k_scale, np.float32).reshape(128, 1))

    su = np.arange(128)[:, None] <= (np.arange(1024)[None, :] - 512)
    maskT = su.astype(BF)                                  # [128, 1024]

    wq2 = np.asarray(wq, np.float32).reshape(D, N * H)
    wk2 = np.asarray(wk, np.float32).reshape(D, KH * H)
    wv2 = np.asarray(wv, np.float32).reshape(D, KH * H)
    wo2 = np.asarray(wo, np.float32)                       # [N, H, D]

    in_maps = []
    for c in range(NCORES):
        wqkv = np.concatenate(
            [
                wq2[:, c * E:(c + 1) * E],
                wk2[:, c * H:(c + 1) * H],
                wv2[:, c * H:(c + 1) * H],
            ],
            axis=1,
        ).astype(BF)                                       # [D, 768]
        wqkvt = np.ascontiguousarray(
            wqkv.reshape(DC, 128, E + 2 * H).transpose(1, 0, 2)
        )                                                  # [128, DC, 768]
        woc = wo2[c * G:(c + 1) * G].astype(BF)            # [G, H, D]
        wot = np.ascontiguousarray(woc.transpose(1, 0, 2))  # [128, G, D]
        in_maps.append(
            {
                "xq": xq,
                "wqkv": wqkvt,
                "wo": wot,
                "cosT": cosT,
                "sinT": sinT,
                "qscale": qsc,
                "kscale": ksc,
                "maskT": maskT,
            }
        )
    return in_maps


def kernel(x, wq, wk, wv, wo, q_scale, k_scale, k_cache, v_cache,
           segment_ids, num_right_pads=0, **_unused):
    from concourse.bass_utils import run_bass_kernel_spmd

    if "nc" not in _CACHE:
        _CACHE["nc"] = _build()
    nc = _CACHE["nc"]

    in_maps = _prep_inputs(x, wq, wk, wv, wo, q_scale, k_scale, segment_ids)
    res = run_bass_kernel_spmd(nc, in_maps, core_ids=list(range(NCORES)))
    total = np.zeros((BT, D), np.float32)
    for c in range(NCORES):
        total += np.asarray(res.results[c]["out"], dtype=np.float32)
    return total.reshape(B, T, D)
